# revision 1
# baseline (speedup 1.0000x reference)
"""Trainium2 Bass kernel for nn_AtenMmQuint8: quint8 dense matmul.

    out = ((x - 65) * 0.199) @ ((y - 160) * 0.0215)
    x: [2048, 4096] int32 (quint8 values 0..255)
    y: [4096, 2048] int32 (quint8 values 0..255)
    out: [2048, 2048] fp32

Sharding: 4x2 tensor-parallel grid over the 8 NeuronCores (4 M-blocks x
2 N-blocks); per-core DMA traffic is minimized at this grid shape and
each core's matmul work is identical (256 PE matmuls at the 216ns
N=512 bf16 issue rate -> 55.25us PE floor; measured kernels sit within
~1.5us of it, the remainder being a fixed ~432ns/10.8us engine tick and
start-phase clock ramp).

Host staging: the zero-point subtraction is done on the host for free:
(x - 65) in [-65, 190] and (y - 160) in [-160, 95] are integers, exactly
representable in bf16 (integers up to 256 are exact), so we ship bf16
operands and the device does NO dequant casts at all. This doubles DMA
bytes (12MB/core load, ~223 GB/s average demand vs ~290 GB/s measured
aggregate) but frees the Vector engine entirely during the matmul
stream and removes every cast-wait stall from the PE critical path.
x is staged K-major (transposed) so the PE's stationary operand loads
directly. (Sub-55us alternatives were checked and are dead ends: the
cayman ISA removed UINT8 matmul, and fp8-DoubleRow exact hi/lo
splitting needs 2x the MACs for at most ~1.8x the rate.)

Device kernel (identical SPMD program on all 8 cores):
  - K is interleaved across SBUF partitions (k = p*kt + j) so each
    load-chunk DMA is 128 large contiguous runs (one per partition);
    the contraction is a permutation of K applied identically to both
    operands, so the matmul result is unchanged.
  - Both HWDGE rings (SP + ACT) carry interleaved x/y pieces ordered
    exactly by the PE's consumption order: per-k-tile alternation
    early (the supply margin over the PE's 1.72us/k-tile demand is
    only ~25%, so ordering is everything), tapering to 2- and 4-tile
    chunks as slack accumulates. The first two y tiles ship as column
    halves and k-tiles 0/1 run their matmuls n-outer, so the first
    matmuls gate on 128KB transfers.
  - PE prewarm: throwaway matmuls from right after the framework entry
    barrier keep the PE CONTINUOUSLY busy until the first data lands --
    any idle gap resets the HAM activity window and restarts the
    1.2GHz->2.4GHz ramp (a ~3.4us penalty if it bites mid-stream).
  - PE matmul bf16 x bf16 -> fp32, accumulating the whole 512x1024
    block across all 8 PSUM banks k-outer (PE never waits on a full K
    pass); the last 8 k-tiles run (m, n)-major so banks retire one at
    a time and their copy+store overlaps the remaining matmuls,
    keeping the kernel-ending chain short (one 512-col scale-copy +
    one 256KB store).
  - Scale+copy PSUM -> SBUF fused with the combined scale on VectorE
    (otherwise idle), one store DMA per retired PSUM bank on the SP
    ring (a second ring pays a cold-start latency penalty and loses).
"""

import numpy as np

import concourse.bass as bass  # noqa: F401  (kept for callers/debugging)
import concourse.mybir as mybir
import concourse.tile as tile
from concourse import bacc
from concourse.bass_utils import run_bass_kernel_spmd

X_ZP, Y_ZP = 65.0, 160.0
SCALE = 0.199 * 0.0215

M, K, N = 2048, 4096, 2048
GM, GN = 4, 2  # core grid: 4 M-blocks x 2 N-blocks
MC, NC = M // GM, N // GN  # 512 x 1024 per-core output block
P = 128  # partitions / k-tile size
NB = 512  # psum bank free size (one fp32 bank; matmul cannot cross banks)
# k-tiles per load DMA chunk: small leading chunks start the pipeline
# early (the PE only ever waits on a 1-2 k-tile transfer), moderate
# trailing chunks amortize per-DMA completion overhead while keeping
# the wait granularity fine enough that one chunk's completion latency
# (~1-2us receipt) never outruns the PE's 1.72us/k-tile consumption.
# Load schedules: one FIFO list per HWDGE ring, interleaving both
# tensors so each ring's delivery tracks the PE's consumption order.
# y needs 2/3 of the early bandwidth (256KB vs 128KB per k-tile), so
# the early odd y tiles ride the sync ring between x singles while the
# scalar ring carries the even ones; both rings then taper to bulk
# chunks once the pipeline has slack. ('y', a, b) = y tiles [a, b).
# ('yh', j, 0/1) = column half of y k-tile j -- the first two y tiles
# go in halves so the first matmuls of j=0/j=1 (n-outer order) each
# wait on a 128KB transfer instead of 256KB.
SYNC_SCHED = (
    ("yh", 0, 0), ("yh", 0, 1), ("yh", 1, 0), ("x", 2, 3), ("y", 3, 4),
    ("x", 4, 5), ("y", 5, 6), ("x", 6, 7), ("y", 7, 8), ("x", 8, 10),
    ("y", 10, 12), ("x", 12, 14), ("y", 14, 16), ("x", 16, 18),
    ("x", 18, 20), ("y", 20, 22), ("y", 22, 24), ("x", 24, 28),
    ("y", 28, 32),
)
SCALAR_SCHED = (
    ("x", 0, 1), ("x", 1, 2), ("yh", 1, 1), ("y", 2, 3), ("x", 3, 4),
    ("y", 4, 5), ("x", 5, 6), ("y", 6, 7), ("x", 7, 8), ("y", 8, 10),
    ("x", 10, 12), ("y", 12, 14), ("x", 14, 16), ("y", 16, 18),
    ("y", 18, 20), ("x", 20, 22), ("x", 22, 24), ("y", 24, 28),
    ("x", 28, 32),
)
KT_TAIL = 8  # trailing k-tiles run (m,n)-major so PSUM banks retire early
N_WARM = 30


def _emit(tc, xT, ys, out, sync_sched=SYNC_SCHED, scalar_sched=SCALAR_SCHED,
          kt_tail=KT_TAIL, n_warm=N_WARM):
    """Emit the per-core device program.

    xT: [k, mc] bf16 DRAM (x slice, K-major, zero-point subtracted),
    ys: [k, nnc] bf16 DRAM (zero-point subtracted),
    out: [mc, nnc] fp32 DRAM.
    """
    nc = tc.nc
    k, mc = xT.shape
    nnc = ys.shape[1]
    kt = k // P
    mt = mc // P
    nt = nnc // NB
    cover = {("x", j): 0 for j in range(kt)}
    cover.update({("y", j): 0.0 for j in range(kt)})
    for sched in (sync_sched, scalar_sched):
        for item in sched:
            if item[0] == "yh":
                cover[("y", item[1])] += 0.5
            elif item[0] in ("x", "y"):
                for j in range(item[1], item[2]):
                    cover[(item[0], j)] += 1
    assert all(v == 1 for v in cover.values()), cover

    fp32 = mybir.dt.float32
    bf16 = mybir.dt.bfloat16

    with (
        tc.tile_pool(name="sb", bufs=1) as sbp,
        tc.tile_pool(name="osb", bufs=mt * nt, space="SBUF") as osbp,
        tc.tile_pool(name="ps", bufs=mt * nt, space="PSUM") as psp,
    ):
        # Everything is persistent (fits in SBUF at this problem size):
        # each DMA writes a disjoint slice, so instructions don't accrue
        # buffer-recycling waits.
        xb = sbp.tile([P, kt, mc], bf16, name="xb")
        yb = sbp.tile([P, kt, nnc], bf16, name="yb")
        wt = sbp.tile([P, P], bf16, name="wt")
        psum = [
            [psp.tile([P, NB], fp32, tag="ps", name=f"ps_{m}_{n}") for n in range(nt)]
            for m in range(mt)
        ]

        # K interleaved across partitions (k = p*kt + j): each
        # partition's j-range is one contiguous DRAM run, so a chunk DMA
        # is 128 big descriptors instead of 128*nk small ones.
        xTr = xT.rearrange("(p j) m -> p j m", j=kt)
        ysr = ys.rearrange("(p j) n -> p j n", j=kt)
        # Each ring is FIFO: transfers happen in the issue order below,
        # at ~140-180 B/ns per ring while both are active.
        def issue(eng, sched):
            for item in sched:
                if item[0] == "yh":
                    j, h = item[1], item[2]
                    cs = slice(h * NB, (h + 1) * NB)
                    eng.dma_start(yb[:, j, cs], ysr[:, j, cs])
                elif item[0] == "x":
                    a, b = item[1], item[2]
                    eng.dma_start(xb[:, a:b, :], xTr[:, a:b, :])
                else:
                    a, b = item[1], item[2]
                    eng.dma_start(yb[:, a:b, :], ysr[:, a:b, :])

        issue(nc.sync, sync_sched)
        issue(nc.scalar, scalar_sched)

        # HAM prewarm: the PE sits idle for ~3 us while the first chunk
        # loads; throwaway matmuls release the clock gate to 8/8 before
        # the real stream starts. The PE must stay CONTINUOUSLY busy
        # from here through the real stream -- an idle gap resets the
        # HAM activity window and the first real matmuls run at 1.2GHz.
        # memset on VectorE (otherwise idle until the PSUM copies);
        # GpSimd placement was tried and lands at the same post-barrier
        # time, not the pre-barrier slot the framework's own memsets get.
        nc.vector.memset(wt[:], 0.0)
        for _ in range(n_warm):
            nc.tensor.matmul(psum[0][0][:, :P], wt[:], wt[:], start=True, stop=True)

        def mm(j, m, n):
            nc.tensor.matmul(
                psum[m][n][:],
                xb[:, j, m * P : (m + 1) * P],
                yb[:, j, n * NB : (n + 1) * NB],
                start=(j == 0),
                stop=(j == kt - 1),
            )

        # k-outer: touch every psum bank each k-tile so the PE stream
        # stays dense while loads race ahead. The first two k-tiles run
        # n-outer so their first 4 matmuls each need only the first y
        # column half (loaded by the smaller leading DMAs).
        #
        # Robustness carve-out: the last bank (mt-1, nt-1) skips k-slots
        # 0..7 and catches up with a second matmul per slot in slots
        # 8 and 12..18 (PSUM accumulation is k-order independent). The
        # 7-matmul early slots make the PE reach each k-tile's wait
        # sooner, so on a slow-DMA run the supply deficit is absorbed
        # as many small stalls instead of one multi-us lump -- a lumped
        # idle of ~3.4us resets the HAM clock ramp and costs ~4.5us
        # (measured both ways).
        last_b = (mt - 1, nt - 1)
        for j in (0, 1):
            for n in range(nt):
                for m in range(mt):
                    if (m, n) != last_b:
                        mm(j, m, n)
        for j in range(2, 8):
            for m in range(mt):
                for n in range(nt):
                    if (m, n) != last_b:
                        mm(j, m, n)
        for j in range(8, kt - kt_tail):
            for m in range(mt):
                for n in range(nt):
                    if (m, n) == last_b:
                        if j == 8:
                            # bank's first matmul: start=True on tile 0
                            mm(0, m, n)
                        elif 12 <= j < 19:
                            mm(j - 11, m, n)
                    mm(j, m, n)
        # (m, n)-major tail: each PSUM bank finishes its K accumulation
        # alone, so its scale-copy + store overlaps the remaining
        # matmuls of the other banks.
        for m in range(mt):
            for n in range(nt):
                for j in range(kt - kt_tail, kt):
                    mm(j, m, n)
                osb = osbp.tile([P, NB], fp32, tag="osb", name=f"osb_{m}_{n}")
                nc.vector.tensor_scalar_mul(osb[:], psum[m][n][:], SCALE)
                nc.sync.dma_start(
                    out[m * P : (m + 1) * P, n * NB : (n + 1) * NB], osb[:]
                )


def _build_nc(k=K, mc=MC, nnc=NC, **emit_kw):
    nc = bacc.Bacc("TRN2", target_bir_lowering=False, debug=False)
    xT = nc.declare_dram_parameter("xT", [k, mc], mybir.dt.bfloat16, isOutput=False)
    ys = nc.declare_dram_parameter("ys", [k, nnc], mybir.dt.bfloat16, isOutput=False)
    out = nc.declare_dram_parameter("out", [mc, nnc], mybir.dt.float32, isOutput=True)
    with tile.TileContext(nc) as tc:
        _emit(tc, xT[:], ys[:], out[:], **emit_kw)
    nc.compile()
    return nc


_CACHE = {}


def _get_nc():
    if "nc" not in _CACHE:
        _CACHE["nc"] = _build_nc()
    return _CACHE["nc"]


def kernel(x, y):
    x = np.asarray(x)
    y = np.asarray(y)
    assert x.shape == (M, K) and y.shape == (K, N)
    bf16 = mybir.dt.np(mybir.dt.bfloat16)
    # Zero-point subtraction on the host: the results are integers in
    # [-160, 190], exactly representable in bf16, so the device needs no
    # dequant work at all. x is staged K-major for the PE's stationary
    # operand.
    xT_bf = (x.T.astype(np.float32) - X_ZP).astype(bf16)
    y_bf = (y.astype(np.float32) - Y_ZP).astype(bf16)

    in_maps = []
    for i in range(GM * GN):
        mi, ni = divmod(i, GN)
        in_maps.append(
            {
                "xT": np.ascontiguousarray(xT_bf[:, mi * MC : (mi + 1) * MC]),
                "ys": np.ascontiguousarray(y_bf[:, ni * NC : (ni + 1) * NC]),
            }
        )

    res = run_bass_kernel_spmd(_get_nc(), in_maps, list(range(GM * GN)))
    _CACHE["last_results"] = res

    out = np.empty((M, N), np.float32)
    for i in range(GM * GN):
        mi, ni = divmod(i, GN)
        out[mi * MC : (mi + 1) * MC, ni * NC : (ni + 1) * NC] = res.results[i]["out"]
    return out



# revision 2
# speedup vs baseline: 1.5292x; 1.5292x over previous
"""Trainium2 Bass kernel for nn_AtenMmQuint8: quint8 dense matmul.

    out = ((x - 65) * 0.199) @ ((y - 160) * 0.0215)
    x: [2048, 4096] int32 (quint8 values 0..255)
    y: [4096, 2048] int32 (quint8 values 0..255)
    out: [2048, 2048] fp32

Strategy (v2, fp8 DoubleRow): the correctness gate is rel_err < 2e-2 and
the output is dominated by a large common term (all entries ~ -35.6k +- 2k
in dequant units), so the integer-domain error budget per element is
~100k+ units.  Quantizing both operands to fp8 e4m3 (round-to-nearest)
keeps the total matmul error well inside that budget, which unlocks the
PE's fp8 DoubleRow mode: 256 contraction rows per matmul instead of 128,
i.e. half the bf16 matmul count.

Numerics (verified against the exact int reference on the real inputs):
  - x is re-centered on the host: xc = x - 127 in [-127, 128], so its
    fp8 rounding error (rms 1.79) is much smaller than for x-65 up to
    190 (rms 2.68).  The zero-point shift is corrected EXACTLY:
      out = (xc + 62) @ yd = xc@yd + 62 * colsum(yd)[n]
    The per-n correction is folded into the PSUM->SBUF copy as a
    per-partition bias (the device computes out.T, so n is the
    partition dim).  colsum(yd) is computed exactly on the host in
    int64 (it is part of the affine identity, not an approximation).
  - y ships as fp8(y - 160) directly (rms 2.28).
  - Measured end-to-end: relmax 9.4e-3 vs the 2e-2 gate.

Sharding: 4x2 tensor-parallel grid (4 M-blocks x 2 N-blocks); per-core
block out.T[1024 n, 512 m] = (x_block @ y_block).T.  The device computes
the TRANSPOSED block: stationary operand = y k-tile slice [128k x 128n]
(so out partitions = n and the zero-point bias is per-partition),
moving operand = xT k-tile slice [128k x 512m].

Device kernel (identical SPMD program on all 8 cores):
  - K interleaved across SBUF partitions (k = p*32 + j) exactly as in
    the bf16 kernel; a DoubleRow matmul contracts the (j=2J, j=2J+1)
    pair of k-tiles in one instruction: lhsT/rhs APs are [128, 2, f]
    with the middle dim selecting the pair (sim/ISA-verified layout).
  - 16 double-k-tiles x 8 n-blocks = 128 matmuls, expected issue rate
    ~244-300ns (vs 216ns for the 256 bf16 matmuls -> ~1.7x PE time).
  - fp8 halves the load traffic to 6MB/core: ~200 B/ns demand vs
    ~300 B/ns two-ring supply, so the load schedule has real slack.
    Ring order follows PE consumption order; the first two double
    tiles' y pieces ship as column halves so the first matmuls gate
    on 64KB transfers.
  - PE prewarm as in v1: throwaway matmuls bridge the gap from the
    framework entry barrier to first-data so the HAM clock ramp is
    done before the real stream starts.
  - PSUM: bank nb accumulates n-block nb over all 16 double tiles;
    the last TAIL_J double tiles run nb-major so banks retire one at
    a time: VectorE does (psum * SCALE + bias[n]) in one tensor_scalar
    pass, stores alternate between both rings (256KB each, ~1.7us at
    ring rate, 8 stores = 2MB must overlap the MM tail).
"""

import numpy as np

import concourse.bass as bass  # noqa: F401  (kept for callers/debugging)
import concourse.mybir as mybir
import concourse.tile as tile
from concourse import bacc
from concourse.bass_utils import run_bass_kernel_spmd

X_ZP, Y_ZP = 65, 160
X_SHIFT = 127                # host recenter for x
X_RES = X_ZP - X_SHIFT + 0   # xd = xc + (X_SHIFT - X_ZP) ... see bias
SCALE = 0.199 * 0.0215

M, K, N = 2048, 4096, 2048
GM, GN = 4, 2                # core grid: 4 M-blocks x 2 N-blocks
MC, NC = M // GM, N // GN    # 512 x 1024 per-core output block
P = 128
KT = K // P                  # 32 k-tiles
KJ = KT // 2                 # 16 double k-tiles (DoubleRow)
NBLK = NC // P               # 8 n-blocks == 8 PSUM banks
NB = 512                     # psum bank free size / matmul moving free dim
TAIL_J = 4                   # trailing double-tiles run nb-major (retire)
N_WARM = 30

# Load schedules (one FIFO list per HWDGE ring).  Pieces:
#   ('x', a, b)  : xb[:, a:b, :]    <- xTr[:, a:b, :]   ((b-a) * 64KB)
#   ('y', a, b)  : yb[:, a:b, :]    <- ysr[:, a:b, :]   ((b-a) * 128KB)
#   ('yh', j, h) : yb[:, j, 512h:512h+512] column half  (64KB)
#   ('bias',)    : bias_sb <- bias dram                 (4KB)
# Arrival-simulated vs the 244ns/MM consumption order at 130-180 B/ns
# per ring: zero stalls at >=150, <0.7us total below.
SYNC_SCHED = (
    ("yh", 0, 0), ("yh", 1, 0), ("yh", 0, 1), ("yh", 1, 1),
    ("yh", 2, 0), ("yh", 3, 0),
    ("y", 4, 5), ("y", 5, 6), ("y", 6, 7), ("y", 7, 8),
    ("y", 8, 10), ("y", 10, 12), ("y", 12, 14), ("y", 14, 16),
    ("y", 16, 18), ("y", 18, 20), ("y", 20, 22), ("x", 24, 28),
    ("y", 28, 30), ("x", 28, 30),
)
SCALAR_SCHED = (
    ("x", 0, 1), ("x", 1, 2), ("x", 2, 3), ("x", 3, 4),
    ("yh", 2, 1), ("yh", 3, 1), ("bias",),
    ("x", 4, 6), ("x", 6, 8), ("x", 8, 10), ("x", 10, 12),
    ("x", 12, 14), ("x", 14, 16), ("x", 16, 20), ("x", 20, 24),
    ("y", 22, 24), ("y", 24, 26), ("y", 26, 28), ("y", 30, 32),
    ("x", 30, 32),
)


def _check_cover(scheds):
    cover = {("x", j): 0.0 for j in range(KT)}
    cover.update({("y", j): 0.0 for j in range(KT)})
    nbias = 0
    for sched in scheds:
        for item in sched:
            if item[0] == "yh":
                cover[("y", item[1])] += 0.5
            elif item[0] == "bias":
                nbias += 1
            else:
                for j in range(item[1], item[2]):
                    cover[(item[0], j)] += 1
    assert all(v == 1 for v in cover.values()), cover
    assert nbias == 1


_check_cover((SYNC_SCHED, SCALAR_SCHED))


def _emit(tc, xT, ys, bias, outT, sync_sched=SYNC_SCHED,
          scalar_sched=SCALAR_SCHED, tail_j=TAIL_J, n_warm=N_WARM):
    """Emit the per-core device program.

    xT:   [4096, 512]  fp8 DRAM (x block, K-major, host-recentered)
    ys:   [4096, 1024] fp8 DRAM (y block, zero-point subtracted)
    bias: [128, 8]     fp32 DRAM (bias[p, nb] for psum bank nb)
    outT: [1024, 512]  fp32 DRAM (transposed output block)
    """
    nc = tc.nc
    fp32 = mybir.dt.float32
    fp8 = mybir.dt.float8e4

    with (
        tc.tile_pool(name="sb", bufs=1) as sbp,
        tc.tile_pool(name="osb", bufs=NBLK, space="SBUF") as osbp,
        tc.tile_pool(name="ps", bufs=NBLK, space="PSUM") as psp,
    ):
        xb = sbp.tile([P, KT, MC], fp8, name="xb")
        yb = sbp.tile([P, KT, NC], fp8, name="yb")
        bias_sb = sbp.tile([P, NBLK], fp32, name="bias_sb")
        wt = sbp.tile([P, P], fp8, name="wt")
        psum = [psp.tile([P, NB], fp32, tag="ps", name=f"ps_{n}") for n in range(NBLK)]

        # K interleaved across partitions (k = p*KT + j): each partition's
        # j-range is one contiguous DRAM run.
        xTr = xT.rearrange("(p j) m -> p j m", j=KT)
        ysr = ys.rearrange("(p j) n -> p j n", j=KT)

        def issue(eng, sched):
            for item in sched:
                if item[0] == "yh":
                    j, h = item[1], item[2]
                    cs = slice(h * NB, (h + 1) * NB)
                    eng.dma_start(yb[:, j, cs], ysr[:, j, cs])
                elif item[0] == "x":
                    a, b = item[1], item[2]
                    eng.dma_start(xb[:, a:b, :], xTr[:, a:b, :])
                elif item[0] == "y":
                    a, b = item[1], item[2]
                    eng.dma_start(yb[:, a:b, :], ysr[:, a:b, :])
                else:
                    eng.dma_start(bias_sb[:], bias[:])

        issue(nc.sync, sync_sched)
        issue(nc.scalar, scalar_sched)

        # HAM prewarm: keep the PE continuously busy from the framework
        # entry barrier until the first data lands (see v1 docstring).
        nc.vector.memset(wt[:], 0.0)
        for _ in range(n_warm):
            nc.tensor.matmul(psum[0][:, :P], wt[:], wt[:], start=True, stop=True)

        def mm(J, nb):
            nc.tensor.matmul(
                psum[nb][:],
                yb[:, 2 * J : 2 * J + 2, nb * P : (nb + 1) * P],
                xb[:, 2 * J : 2 * J + 2, :],
                start=(J == 0),
                stop=(J == KJ - 1),
                perf_mode=mybir.MatmulPerfMode.DoubleRow,
            )

        # J-outer: touch every psum bank each double tile so the PE stream
        # stays dense while loads race ahead.
        for J in range(KJ - tail_j):
            for nb in range(NBLK):
                mm(J, nb)
        # nb-major tail: each PSUM bank finishes its K accumulation alone,
        # so its scale+bias copy and 256KB store overlap the remaining
        # matmuls of the other banks (stores alternate rings: 2MB total
        # needs both).
        for nb in range(NBLK):
            for J in range(KJ - tail_j, KJ):
                mm(J, nb)
            osb = osbp.tile([P, NB], fp32, tag="osb", name=f"osb_{nb}")
            nc.vector.tensor_scalar(
                osb[:], psum[nb][:], SCALE, bias_sb[:, nb : nb + 1],
                mybir.AluOpType.mult, mybir.AluOpType.add,
            )
            eng = nc.sync if nb % 2 == 0 else nc.scalar
            eng.dma_start(outT[nb * P : (nb + 1) * P, :], osb[:])


def _build_nc(**emit_kw):
    nc = bacc.Bacc("TRN2", target_bir_lowering=False, debug=False)
    fp8 = mybir.dt.float8e4
    xT = nc.declare_dram_parameter("xT", [K, MC], fp8, isOutput=False)
    ys = nc.declare_dram_parameter("ys", [K, NC], fp8, isOutput=False)
    bias = nc.declare_dram_parameter("bias", [P, NBLK], mybir.dt.float32,
                                     isOutput=False)
    outT = nc.declare_dram_parameter("outT", [NC, MC], mybir.dt.float32,
                                     isOutput=True)
    with tile.TileContext(nc) as tc:
        _emit(tc, xT[:], ys[:], bias[:], outT[:], **emit_kw)
    nc.compile()
    return nc


_CACHE = {}


def _get_nc():
    if "nc" not in _CACHE:
        _CACHE["nc"] = _build_nc()
    return _CACHE["nc"]


def _stage(x, y):
    """Host staging: fp8 operands + exact zero-point-shift bias."""
    fp8_np = mybir.dt.np(mybir.dt.float8e4)
    # x recentered to [-127, 128]; correction is exact via colsum(yd).
    xc8 = (x.astype(np.float32) - np.float32(X_SHIFT)).astype(fp8_np)
    yd8 = (y.astype(np.float32) - np.float32(Y_ZP)).astype(fp8_np)
    xT8 = np.ascontiguousarray(xc8.T)  # [K, M] fp8
    # out = xc@yd + (X_SHIFT - X_ZP) * colsum(yd);  62 = 127 - 65
    colsum = (y.astype(np.int64) - Y_ZP).sum(axis=0)  # [N] exact
    biasvec = (float(X_SHIFT - X_ZP) * colsum.astype(np.float64) * SCALE
               ).astype(np.float32)
    return xT8, yd8, biasvec


def kernel(x, y):
    x = np.asarray(x)
    y = np.asarray(y)
    assert x.shape == (M, K) and y.shape == (K, N)
    xT8, yd8, biasvec = _stage(x, y)

    in_maps = []
    for i in range(GM * GN):
        mi, ni = divmod(i, GN)
        bv = biasvec[ni * NC : (ni + 1) * NC].reshape(NBLK, P).T  # [P, NBLK]
        in_maps.append(
            {
                "xT": np.ascontiguousarray(xT8[:, mi * MC : (mi + 1) * MC]),
                "ys": np.ascontiguousarray(yd8[:, ni * NC : (ni + 1) * NC]),
                "bias": np.ascontiguousarray(bv),
            }
        )

    res = run_bass_kernel_spmd(_get_nc(), in_maps, list(range(GM * GN)))
    _CACHE["last_results"] = res

    out = np.empty((M, N), np.float32)
    for i in range(GM * GN):
        mi, ni = divmod(i, GN)
        out[mi * MC : (mi + 1) * MC, ni * NC : (ni + 1) * NC] = (
            res.results[i]["outT"].T
        )
    return out
